# revision 8
# baseline (speedup 1.0000x reference)
"""Trainium2 Bass kernel for nn_MultiHeadAttn_80126909874682.

Full MHA layer: QKV projection -> 16-head attention (seq 2048) -> output
projection -> residual -> LayerNorm, over h [2048, 4, 1024] fp32.

Sharding (8 NeuronCores, zero collectives):
  core c -> batch b = c // 2, token-half r = c % 2.
  Each core computes K/V for all 2048 tokens of its batch (all 16 heads)
  and Q / attention / output projection / LayerNorm for its 1024 local
  tokens only.  The per-core `hb` input is permuted so the core's local
  tokens come first; attention is invariant to the j-permutation of K/V,
  so the program stays uniform SPMD while the data differs per core.

Compute dtypes: matmul operands bf16 (weights pre-converted on host),
PSUM accumulation / softmax statistics / LayerNorm in fp32.
Softmax uses exp without max-subtraction (scores are O(1) by
construction) and a ones-column appended to V so the PV matmul also
produces the softmax denominators.
"""

import sys

sys.path.insert(0, "/opt/trn_rl_repo")

import numpy as np
import ml_dtypes

import concourse.bass as bass
import concourse.tile as tile
from concourse import bacc, mybir
from concourse.bass import ts
from concourse.bass_utils import run_bass_kernel_spmd
from concourse.masks import make_identity

N_HEAD = 16
D_MODEL = 1024
D_HEAD = 64
SEQ = 2048
BATCH = 4
EPS = 1e-5
N_CORES = 8

LOCAL = SEQ // 2            # tokens owned per core (1024)
N_PAIR = N_HEAD // 2        # head pairs (8)
CC = D_MODEL // 128         # contraction chunks (8)
JT = SEQ // 128             # j tiles (16)
IT_ALL = SEQ // 128         # token tiles for transpose (16)
IB_ALL = SEQ // 512         # 512-token blocks, all tokens (4)
IB_LOC = LOCAL // 512       # 512-token blocks, local tokens (2)
ISUB = LOCAL // 128         # 128-token sub tiles, local (8)

F32 = mybir.dt.float32
BF16 = mybir.dt.bfloat16
F32R = mybir.dt.float32r
AF = mybir.ActivationFunctionType


def _r(ap):
    """View an fp32 AP as float32r for full-rate TensorEngine streaming."""
    return ap.bitcast(F32R)


def build_program():
    nc = bacc.Bacc()

    hb = nc.declare_dram_parameter("hb", [SEQ, D_MODEL], F32, isOutput=False)
    wq = nc.declare_dram_parameter("wq", [D_MODEL, D_MODEL], BF16, isOutput=False)
    wk = nc.declare_dram_parameter("wk", [D_MODEL, D_MODEL], BF16, isOutput=False)
    wv = nc.declare_dram_parameter("wv", [D_MODEL, D_MODEL], BF16, isOutput=False)
    wo = nc.declare_dram_parameter("wo", [D_MODEL, D_MODEL], BF16, isOutput=False)
    gamma = nc.declare_dram_parameter("gamma", [D_MODEL], F32, isOutput=False)
    beta = nc.declare_dram_parameter("beta", [D_MODEL], F32, isOutput=False)
    out = nc.declare_dram_parameter("out", [LOCAL, D_MODEL], F32, isOutput=True)

    with tile.TileContext(nc) as tc:
        with (
            tc.tile_pool(name="consts", bufs=1) as consts,
            tc.tile_pool(name="wo_w", bufs=1) as wo_pool,
            tc.tile_pool(name="qkv", bufs=1) as qkv_pool,
            tc.tile_pool(name="psum", bufs=8, space="PSUM") as psum,
        ):
            # ---- constants ----
            ident = consts.tile([128, 128], F32)
            make_identity(nc, ident[:])
            ones64 = consts.tile([1, 64], F32)
            nc.vector.memset(ones64[:], 1.0)
            gamma_b = consts.tile([128, D_MODEL], F32)
            beta_b = consts.tile([128, D_MODEL], F32)
            g_ap, b_ap = gamma.ap(), beta.ap()
            nc.gpsimd.dma_start(
                out=gamma_b[:],
                in_=bass.AP(tensor=g_ap.tensor, offset=g_ap.offset,
                            ap=[[0, 128], [1, D_MODEL]]),
            )
            nc.gpsimd.dma_start(
                out=beta_b[:],
                in_=bass.AP(tensor=b_ap.tensor, offset=b_ap.offset,
                            ap=[[0, 128], [1, D_MODEL]]),
            )
            eps_t = consts.tile([128, 1], F32)
            nc.vector.memset(eps_t[:], EPS)
            mean_all = consts.tile([128, ISUB], F32)
            var_all = consts.tile([128, ISUB], F32)

            wo_sb = [wo_pool.tile([128, D_MODEL], BF16, tag=f"wo{c}", name=f"wo{c}")
                     for c in range(CC)]
            for c in range(CC):
                nc.sync.dma_start(wo_sb[c][:], wo[ts(c, 128), :])

            # long-lived projection outputs
            qt = [qkv_pool.tile([128, LOCAL], BF16, tag=f"qt{p}", name=f"qt{p}")
                  for p in range(N_PAIR)]
            kt = [qkv_pool.tile([128, SEQ], BF16, tag=f"kt{p}", name=f"kt{p}")
                  for p in range(N_PAIR)]
            v_sb = [qkv_pool.tile([128, JT * 65], BF16, tag=f"v{n}", name=f"v{n}")
                    for n in range(N_HEAD)]

            # ---- phase 1: transpose h, project Q/K/V (transient pools) ----
            with (
                tc.tile_pool(name="hbt", bufs=1) as hbt_pool,
                tc.tile_pool(name="hrow", bufs=2) as hrow_pool,
            ):
                # hb^T (bf16) via PE transpose
                hbt = [hbt_pool.tile([128, SEQ], BF16, tag=f"hbt{c}", name=f"hbt{c}")
                       for c in range(CC)]
                for it in range(IT_ALL):
                    hrow = hrow_pool.tile([128, D_MODEL], F32, tag="hrow")
                    nc.sync.dma_start(hrow[:], hb[ts(it, 128), :])
                    for c in range(CC):
                        tp = psum.tile([128, 128], F32, tag="ps")
                        nc.tensor.transpose(tp[:], hrow[:, ts(c, 128)], ident[:])
                        nc.vector.tensor_copy(hbt[c][:, ts(it, 128)], tp[:])

                # V (all tokens) + ones column per head
                with tc.tile_pool(name="w_v", bufs=1) as wv_pool:
                    wv_sb = [wv_pool.tile([128, D_MODEL], BF16, tag=f"wv{c}",
                                          name=f"wv{c}") for c in range(CC)]
                    for c in range(CC):
                        nc.sync.dma_start(wv_sb[c][:], wv[ts(c, 128), :])
                    for n in range(N_HEAD):
                        nc.gpsimd.memset(v_sb[n][:], 1.0)
                    for j in range(JT):
                        for half in range(2):
                            ps = psum.tile([128, 512], F32, tag="ps")
                            for c in range(CC):
                                nc.tensor.matmul(
                                    ps[:], hbt[c][:, ts(j, 128)],
                                    wv_sb[c][:, ts(half, 512)],
                                    start=(c == 0), stop=(c == CC - 1),
                                )
                            for k in range(8):
                                n = 8 * half + k
                                nc.vector.tensor_copy(
                                    v_sb[n][:, j * 65: j * 65 + 64],
                                    ps[:, ts(k, 64)]
                                )

                # K^T (all tokens), Q^T (local tokens): pair-tiles [128, tok]
                with tc.tile_pool(name="w_qk", bufs=1) as wqk_pool:
                    wq_sb = [wqk_pool.tile([128, D_MODEL], BF16, tag=f"wq{c}",
                                           name=f"wq{c}") for c in range(CC)]
                    wk_sb = [wqk_pool.tile([128, D_MODEL], BF16, tag=f"wk{c}",
                                           name=f"wk{c}") for c in range(CC)]
                    for c in range(CC):
                        nc.sync.dma_start(wq_sb[c][:], wq[ts(c, 128), :])
                        nc.sync.dma_start(wk_sb[c][:], wk[ts(c, 128), :])
                    for p in range(N_PAIR):
                        for ib in range(IB_ALL):
                            ps = psum.tile([128, 512], F32, tag="ps")
                            for c in range(CC):
                                nc.tensor.matmul(
                                    ps[:], wk_sb[c][:, ts(p, 128)],
                                    hbt[c][:, ts(ib, 512)],
                                    start=(c == 0), stop=(c == CC - 1),
                                )
                            nc.vector.tensor_copy(kt[p][:, ts(ib, 512)], ps[:])
                        for ib in range(IB_LOC):
                            ps = psum.tile([128, 512], F32, tag="ps")
                            for c in range(CC):
                                nc.tensor.matmul(
                                    ps[:], wq_sb[c][:, ts(p, 128)],
                                    hbt[c][:, ts(ib, 512)],
                                    start=(c == 0), stop=(c == CC - 1),
                                )
                            nc.vector.tensor_copy(qt[p][:, ts(ib, 512)], ps[:])

            # ---- phases 2+3: attention, output projection, LayerNorm ----
            import contextlib
            with contextlib.ExitStack() as ph2:
                attn_pool = ph2.enter_context(tc.tile_pool(name="attnT", bufs=2))
                exp_pool = ph2.enter_context(tc.tile_pool(name="exp", bufs=6))
                rec_pool = ph2.enter_context(tc.tile_pool(name="small", bufs=4))
                x_pool = ph2.enter_context(tc.tile_pool(name="xres", bufs=1))
                hbr_pool = ph2.enter_context(tc.tile_pool(name="hbres", bufs=3))
                io_pool = ph2.enter_context(tc.tile_pool(name="io", bufs=2))
                _phase23(nc, tc, psum, consts, locals())

    nc.finalize()
    return nc


def _phase23(nc, tc, psum, consts, env):
    (attn_pool, exp_pool, rec_pool, x_pool, hbr_pool, io_pool) = (
        env["attn_pool"], env["exp_pool"], env["rec_pool"],
        env["x_pool"], env["hbr_pool"], env["io_pool"])
    qt, kt, v_sb, wo_sb = env["qt"], env["kt"], env["v_sb"], env["wo_sb"]
    hb, out = env["hb"], env["out"]
    ones64, gamma_b, beta_b, eps_t = (env["ones64"], env["gamma_b"],
                                      env["beta_b"], env["eps_t"])
    mean_all, var_all = env["mean_all"], env["var_all"]

    if True:
        if True:
            xres = [x_pool.tile([128, D_MODEL], F32, tag=f"x{i}", name=f"x{i}")
                    for i in range(ISUB)]

            for itile in range(IB_LOC):
                at = [attn_pool.tile([128, 512], BF16, tag=f"at{p}", name=f"at{p}")
                      for p in range(N_PAIR)]
                for p in range(N_PAIR):
                    acc = [psum.tile([128, 512], F32, tag="ps", name="acc") for _ in range(2)]
                    for jc in range(JT):
                        for h in range(2):
                            n = 2 * p + h
                            sp = psum.tile([128, 512], F32, tag="ps")
                            nc.tensor.matmul(
                                sp[:],
                                kt[p][ts(h, 64), ts(jc, 128)],
                                qt[p][ts(h, 64), ts(itile, 512)],
                                start=True, stop=True,
                            )
                            e = exp_pool.tile([128, 512], BF16, tag="e")
                            nc.scalar.activation(e[:], sp[:], AF.Exp)
                            nc.tensor.matmul(
                                acc[h][0:65, :],
                                v_sb[n][:, jc * 65: jc * 65 + 65], e[:],
                                start=(jc == 0), stop=(jc == JT - 1),
                            )
                    for h in range(2):
                        rec = rec_pool.tile([1, 512], F32, tag="rec")
                        nc.vector.reciprocal(rec[:], acc[h][64:65, :])
                        rb = rec_pool.tile([64, 512], F32, tag="recb")
                        nc.gpsimd.partition_broadcast(rb[:], rec[:])
                        nc.vector.tensor_mul(
                            at[p][ts(h, 64), :], acc[h][0:64, :], rb[:]
                        )
                # output projection + residual for this 512-token block
                for s4 in range(4):
                    isub = 4 * itile + s4
                    hbres = hbr_pool.tile([128, D_MODEL], F32, tag="hbres")
                    nc.sync.dma_start(hbres[:], hb[ts(isub, 128), :])
                    for dm in range(2):
                        ops = psum.tile([128, 512], F32, tag="ps")
                        for p in range(N_PAIR):
                            nc.tensor.matmul(
                                ops[:], at[p][:, ts(s4, 128)],
                                wo_sb[p][:, ts(dm, 512)],
                                start=(p == 0), stop=(p == N_PAIR - 1),
                            )
                        nc.vector.tensor_add(
                            xres[isub][:, ts(dm, 512)], ops[:],
                            hbres[:, ts(dm, 512)],
                        )
                    # LayerNorm stats (free-axis, 2 subgroups of 512)
                    stats = rec_pool.tile([128, 2, 6], F32, tag="bnst")
                    mv = rec_pool.tile([128, 2], F32, tag="bnmv")
                    for g in range(2):
                        nc.vector.bn_stats(stats[:, g, :], xres[isub][:, ts(g, 512)])
                    nc.vector.bn_aggr(mv[:], stats[:])
                    nc.vector.tensor_copy(mean_all[:, isub: isub + 1], mv[:, 0:1])
                    nc.vector.tensor_copy(var_all[:, isub: isub + 1], mv[:, 1:2])

            # ---- phase 3: LayerNorm tail ----
            rstd = consts.tile([128, ISUB], F32)
            nc.scalar.activation(rstd[:], var_all[:], AF.Sqrt, bias=eps_t[:])
            nc.vector.reciprocal(rstd[:], rstd[:])
            for isub in range(ISUB):
                xc = io_pool.tile([128, D_MODEL], F32, tag="xc")
                nc.vector.tensor_scalar(
                    xc[:], xres[isub][:],
                    mean_all[:, isub: isub + 1], rstd[:, isub: isub + 1],
                    op0=mybir.AluOpType.subtract, op1=mybir.AluOpType.mult,
                )
                nc.vector.tensor_mul(xc[:], xc[:], gamma_b[:])
                nc.vector.tensor_add(xc[:], xc[:], beta_b[:])
                nc.sync.dma_start(out[ts(isub, 128), :], xc[:])


_program_cache = {}


def _get_program():
    if "nc" not in _program_cache:
        _program_cache["nc"] = build_program()
    return _program_cache["nc"]


def _shard_inputs(h, Wq, Wkv, Wo, gamma, beta):
    """Build the 8 per-core input maps (host-side numpy only)."""
    h = np.asarray(h, np.float32)
    Wq = np.asarray(Wq, np.float32)
    Wkv = np.asarray(Wkv, np.float32)
    Wo = np.asarray(Wo, np.float32)
    gamma = np.asarray(gamma, np.float32)
    beta = np.asarray(beta, np.float32)

    scale = 1.0 / np.sqrt(D_HEAD)
    Wq_s = np.ascontiguousarray((Wq * scale).astype(ml_dtypes.bfloat16))
    Wk = np.ascontiguousarray(Wkv[:, :N_HEAD * D_HEAD].astype(ml_dtypes.bfloat16))
    Wv = np.ascontiguousarray(Wkv[:, N_HEAD * D_HEAD:].astype(ml_dtypes.bfloat16))
    Wo_b = np.ascontiguousarray(Wo.astype(ml_dtypes.bfloat16))

    in_maps = []
    for core in range(N_CORES):
        b, r = divmod(core, 2)
        hb_full = h[:, b, :]  # [2048, 1024]
        if r == 0:
            hb_perm = hb_full
        else:
            hb_perm = np.concatenate([hb_full[LOCAL:], hb_full[:LOCAL]], axis=0)
        in_maps.append({
            "hb": np.ascontiguousarray(hb_perm),
            "wq": Wq_s, "wk": Wk, "wv": Wv, "wo": Wo_b,
            "gamma": gamma, "beta": beta,
        })
    return in_maps


def kernel(h, Wq, Wkv, Wo, gamma, beta, _trace=False):
    nc = _get_program()
    in_maps = _shard_inputs(h, Wq, Wkv, Wo, gamma, beta)
    res = run_bass_kernel_spmd(nc, in_maps, list(range(N_CORES)), trace=_trace)
    if _trace:
        kernel.last_results = res

    out = np.empty((SEQ, BATCH, D_MODEL), np.float32)
    for core in range(N_CORES):
        b, r = divmod(core, 2)
        out[r * LOCAL:(r + 1) * LOCAL, b, :] = res.results[core]["out"]
    return out
